# revision 21
# baseline (speedup 1.0000x reference)
"""AutoDisBucketEncoder Trainium2 kernel (8 NeuronCores, feature-sharded).

Math (per feature f, batch b):
  h = lrelu(x_aug @ w1_aug)            # bias folded via ones column
  h = lrelu(h @ (rw_l + I) + rb_l)     # x3, residual folded into weights
  z = lrelu(h @ w2 + b2)
  e = exp(z * tau)
  out = (e / sum_k e) @ emb

Layout: features sharded 32/core; each core packs 2 features per 128
partitions (block-diagonal weights), streams the full 2048 batch as the
matmul moving dim.  Softmax runs in [k, b] layout; the sum-over-k and its
broadcast back to 128 partitions are done by one ones-block matmul.  The
embedding matmul packs 2 pairs (4 features) per instruction: stationary =
normalized probs [64, 128-batch], moving = block-diagonal emb [64, 512]
with zeroed garbage stripes; the two pair-groups sit on different PE
row-halves so their matmuls run concurrently, and each PSUM tile lands in
[batch, 8-feature] layout so output DMAs write 2KB contiguous lines.

All leaky-relus use Prelu (parametric_relu), which shares the activation
table set with Exp — no ACT_TABLE_LOAD switching mid-kernel.
"""

import sys

sys.path.insert(0, "/opt/trn_rl_repo")

import numpy as np
import ml_dtypes
from contextlib import ExitStack

BF16 = ml_dtypes.bfloat16
B, F, D, K, E = 2048, 256, 64, 8, 128
NCORES = 8
FC = F // NCORES          # 32 features per core
NPAIR = FC // 2           # 16
NSTACK = NPAIR // 4       # 4 stacks of 4 pairs
NEG = 0.01                # leaky slope
HB = B // 2               # 1024 batch half-chunk (2 PSUM banks in f32)

# h-chunk evictions: idx % MOD >= LIM -> DVE custom-op path, else ACT.
# idx % 4 is the pair index within a step; odd pairs go to DVE so each
# group's two evictions run on different engines concurrently.
DVE_RES_MOD = 2
DVE_RES_LIM = 1
# emb-psum evictions: idx % 16 < 7 -> ACT copy, else DVE copy
EMB_ACT_MOD = 16
EMB_ACT_LIM = 7

_compiled = None
SIM_SAFE = False  # substitute Relu for Prelu so CoreSim can execute


def _register_leaky_bias():
    import numpy as np
    from concourse.dve_spec import Spec, Src0, C0, C1, maxx, lower
    from concourse.dve_ops import (
        DveOp, DveOpSpec, OPS, CUSTOM_DVE_SPECS, _SUB_OPCODE_FOR_NAME,
        _CUSTOM_DVE_ROW_BASE, has_src1,
    )

    if "LEAKY_BIAS_ANT" in CUSTOM_DVE_SPECS:
        return next(o for o in OPS if o.name == "LEAKY_BIAS_ANT")
    spec = Spec(
        body=maxx(Src0 + C0, (Src0 + C0) * C1),
        reference=lambda in0, in1, s0, s1, imm2: np.maximum(
            in0 + s0, (in0 + s0) * s1
        ).astype(np.float32),
    )
    row = _CUSTOM_DVE_ROW_BASE + len(OPS)
    shas = {}
    for ver in ("v3", "v4"):
        uops = lower(spec, ver=ver)
        shas[ver] = DveOpSpec(
            name="LEAKY_BIAS_ANT", opcode=row, uops=uops, rd1_en=has_src1(spec)
        ).sha(ver)
    op = DveOp("LEAKY_BIAS_ANT", spec, subdim=False, uops_sha=shas)
    OPS.append(op)
    CUSTOM_DVE_SPECS[op.name] = spec
    _SUB_OPCODE_FOR_NAME[op.name] = row
    return op


def _build_bass():
    import concourse.bass as bass  # noqa: F401
    import concourse.mybir as mybir
    import concourse.tile as tile
    from concourse import bacc

    LEAKY_OP = _register_leaky_bias()

    dt = mybir.dt
    AF = mybir.ActivationFunctionType
    LRELU = AF.Relu if SIM_SAFE else AF.Prelu

    nc = bacc.Bacc("TRN2", target_bir_lowering=False, debug=False)

    xp = nc.dram_tensor("xp", [NPAIR, 8, B], dt.bfloat16, kind="ExternalInput").ap()
    w1p = nc.dram_tensor("w1p", [64, NPAIR * 128], dt.bfloat16, kind="ExternalInput").ap()
    rwp = nc.dram_tensor("rwp", [128, 3 * NPAIR * 128], dt.bfloat16, kind="ExternalInput").ap()
    rbp = nc.dram_tensor("rbp", [128, 3 * NPAIR], dt.float32, kind="ExternalInput").ap()
    w2p = nc.dram_tensor("w2p", [128, NPAIR * 32], dt.bfloat16, kind="ExternalInput").ap()
    b2s = nc.dram_tensor("b2s", [128, NSTACK], dt.float32, kind="ExternalInput").ap()
    taus = nc.dram_tensor("taus", [128, NSTACK], dt.float32, kind="ExternalInput").ap()
    onesbd = nc.dram_tensor("onesbd", [128, 128], dt.bfloat16, kind="ExternalInput").ap()
    e4p = nc.dram_tensor("e4p", [128, NSTACK * 512], dt.bfloat16, kind="ExternalInput").ap()
    out = nc.dram_tensor("out", [B, FC * E], dt.bfloat16, kind="ExternalOutput").ap()

    with tile.TileContext(nc) as tc, ExitStack() as ctx:
        const = ctx.enter_context(tc.tile_pool(name="const", bufs=1))
        xpool = ctx.enter_context(tc.tile_pool(name="xpool", bufs=3))
        hpool = ctx.enter_context(tc.tile_pool(name="hpool", bufs=8))
        tpool = ctx.enter_context(tc.tile_pool(name="tpool", bufs=3))
        epool = ctx.enter_context(tc.tile_pool(name="epool", bufs=2))
        rpool = ctx.enter_context(tc.tile_pool(name="rpool", bufs=2))
        opool = ctx.enter_context(tc.tile_pool(name="opool", bufs=4))
        h_ps = ctx.enter_context(tc.tile_pool(name="h_ps", bufs=3, space="PSUM"))
        po_ps = ctx.enter_context(tc.tile_pool(name="po_ps", bufs=2, space="PSUM"))

        # ---- constants into SBUF (all host-side pre-transposed, contiguous) ----
        w1_sb = const.tile([64, NPAIR * 128], dt.bfloat16)
        nc.sync.dma_start(out=w1_sb, in_=w1p)
        rb_sb = const.tile([128, 3 * NPAIR], dt.float32)
        nc.sync.dma_start(out=rb_sb, in_=rbp)
        rw_sb = const.tile([128, 3 * NPAIR * 128], dt.bfloat16)
        for l in range(3):
            nc.sync.dma_start(
                out=rw_sb[:, l * NPAIR * 128 : (l + 1) * NPAIR * 128],
                in_=rwp[:, l * NPAIR * 128 : (l + 1) * NPAIR * 128],
            )
        w2_sb = const.tile([128, NPAIR * 32], dt.bfloat16)
        nc.sync.dma_start(out=w2_sb, in_=w2p)
        b2_sb = const.tile([128, NSTACK], dt.float32)
        nc.sync.dma_start(out=b2_sb, in_=b2s)
        tau_sb = const.tile([128, NSTACK], dt.float32)
        nc.sync.dma_start(out=tau_sb, in_=taus)
        ones_sb = const.tile([128, 128], dt.bfloat16)
        nc.sync.dma_start(out=ones_sb, in_=onesbd)
        e4_sb = const.tile([128, NSTACK * 512], dt.bfloat16)
        nc.sync.dma_start(out=e4_sb, in_=e4p)

        # out[b, fc*E] viewed as [bc(16), p(128), s(4), e(1024)]
        out_r = out.rearrange("(bc p) (s e) -> bc s p e", p=128, e=1024)

        def evict_h(idx, h, ph, rb_ap):
            """psum -> sbuf bf16 with (optional bias add and) leaky relu."""
            if idx % DVE_RES_MOD >= DVE_RES_LIM:
                nc.vector._custom_dve(
                    LEAKY_OP,
                    out=h,
                    in0=ph,
                    s0=0.0 if rb_ap is None else rb_ap,
                    s1=NEG,
                )
            else:
                if rb_ap is None:
                    nc.scalar.activation(h, ph, LRELU, alpha=NEG)
                else:
                    nc.scalar.activation(h, ph, LRELU, bias=rb_ap, alpha=NEG)

        res_idx = 0
        pending_z = []     # deferred z-epilogue (dual exp + sumexp mms)
        pending_tail = []  # deferred DVE softmax tail (recip/mul)
        pending_emb = []   # emb po-group closures

        def emit(lst, n=99):
            for _ in range(min(n, len(lst))):
                lst.pop(0)()

        for s in range(NSTACK):
            e_sb = epool.tile([128, B], dt.bfloat16, tag="e", name=f"e{s}")
            en_sb = epool.tile([128, B], dt.bfloat16, tag="en", name=f"en{s}")
            for c in range(2):
                # ---- h pipeline: pairs processed two at a time; the even
                # pair uses the diagonal 64x64 PE tiles and the odd pair the
                # off-diagonal ones, so each residual step runs 4 concurrent
                # matmuls (full PE array).  The odd pair's two features swap
                # partition halves every residual layer (ping-pong); host
                # packing of rw/rb/w2 accounts for it.  Previous chunk's
                # softmax/emb work drips in between steps. ----
                hs = [None] * 4
                for grp in range(2):
                    xt = xpool.tile(
                        [64, HB], dt.bfloat16, tag="x", name=f"x{s}_{grp}_{c}"
                    )
                    for jj in range(2):
                        p = 4 * s + 2 * grp + jj
                        # inputs go out on the GpSimd DMA queue so they never
                        # queue behind output DMAs that wait on evictions
                        nc.gpsimd.dma_start(
                            out=xt[32 * jj : 32 * jj + 8, :],
                            in_=xp[p][:, c * HB : (c + 1) * HB],
                        )
                    hs[2 * grp] = xt
                    hs[2 * grp + 1] = xt
                for step in range(4):  # 0: L1, 1-3: residual layers
                    for grp in range(2):
                        phs = [None, None]
                        for jj in range(2):
                            j = 2 * grp + jj
                            p = 4 * s + j
                            phs[jj] = h_ps.tile(
                                [128, HB], dt.float32, tag="h",
                                name=f"ph{p}_{c}_{step}",
                            )
                        if step == 0:
                            for q in range(2):
                                qs = slice(q * 512, (q + 1) * 512)
                                for jj in range(2):
                                    p = 4 * s + 2 * grp + jj
                                    r0 = 32 * jj
                                    nc.tensor.matmul(
                                        phs[jj][:, qs],
                                        w1_sb[r0 : r0 + 8, p * 128 : (p + 1) * 128],
                                        hs[2 * grp + jj][r0 : r0 + 8, qs],
                                        start=True,
                                        stop=True,
                                    )
                        else:
                            l = step - 1
                            for q in range(2):
                                qs = slice(q * 512, (q + 1) * 512)
                                for jj in range(2):
                                    p = 4 * s + 2 * grp + jj
                                    for fi in range(2):
                                        if jj == 0:
                                            rin = ro = 64 * fi
                                        else:
                                            rin = 64 * ((l + fi) % 2)
                                            ro = 64 * ((l + 1 + fi) % 2)
                                        off = ((l * NPAIR + p) * 2 + fi) * 64
                                        nc.tensor.matmul(
                                            phs[jj][ro : ro + 64, qs],
                                            rw_sb[rin : rin + 64, off : off + 64],
                                            hs[2 * grp + jj][rin : rin + 64, qs],
                                            start=True,
                                            stop=True,
                                        )
                        for jj in range(2):
                            j = 2 * grp + jj
                            p = 4 * s + j
                            rb_ap = (
                                None
                                if step == 0
                                else rb_sb[
                                    :, (step - 1) * NPAIR + p : (step - 1) * NPAIR + p + 1
                                ]
                            )
                            h2 = hpool.tile(
                                [128, HB], dt.bfloat16, tag="h",
                                name=f"h{p}_{c}_{step}",
                            )
                            evict_h(res_idx, h2, phs[jj], rb_ap)
                            res_idx += 1
                            hs[j] = h2
                    if step == 0:
                        emit(pending_z)
                    elif step == 1:
                        emit(pending_tail)
                    else:
                        emit(pending_emb, 3)
                pzs = [
                    po_ps.tile([128, 512], dt.float32, tag="po", name=f"pz{s}_{c}_{q}")
                    for q in range(2)
                ]
                for q in range(2):
                    for j in range(4):
                        p = 4 * s + j
                        nc.tensor.matmul(
                            pzs[q][32 * j : 32 * j + 32, :],
                            w2_sb[:, p * 32 : (p + 1) * 32],
                            hs[j][:, q * 512 : (q + 1) * 512],
                            start=True,
                            stop=True,
                            tile_position=(0, 32 * j),
                        )
                emit(pending_emb)

                def make_z(s_, c_, pz_refs, e_ref, en_ref, last_):
                    def z_chain():
                        t1 = tpool.tile(
                            [128, HB], dt.float32, tag="zt", name=f"t1_{s_}_{c_}"
                        )
                        for q in range(2):
                            nc.scalar.activation(
                                t1[:, q * 512 : (q + 1) * 512], pz_refs[q], LRELU,
                                bias=b2_sb[:, s_ : s_ + 1], alpha=NEG,
                            )
                        ev = e_ref[:, c_ * HB : (c_ + 1) * HB]
                        nc.scalar.activation(
                            ev, t1, AF.Exp, scale=tau_sb[:, s_ : s_ + 1]
                        )
                        pss = [
                            po_ps.tile(
                                [128, 512], dt.float32, tag="po",
                                name=f"psum{s_}_{c_}_{q}",
                            )
                            for q in range(2)
                        ]
                        for q in range(2):
                            nc.tensor.matmul(
                                pss[q],
                                ones_sb,
                                ev[:, q * 512 : (q + 1) * 512],
                                start=True,
                                stop=True,
                            )

                        def tail():
                            rcf = rpool.tile(
                                [128, HB], dt.float32, tag="rcf", name=f"rcf{s_}_{c_}"
                            )
                            for q in range(2):
                                nc.vector.reciprocal_approx_fast(
                                    out=rcf[:, q * 512 : (q + 1) * 512], in_=pss[q]
                                )
                            eng = nc.vector if last_ else nc.gpsimd
                            eng.tensor_mul(
                                en_ref[:, c_ * HB : (c_ + 1) * HB], ev, rcf
                            )

                        pending_tail.append(tail)

                    return z_chain

                pending_z.append(
                    make_z(s, c, pzs, e_sb, en_sb, s == NSTACK - 1 and c == 1)
                )

                def make_group(s_, bc2_, en_ref):
                    def emit_group():
                        ob = opool.tile(
                            [128, 1024], dt.bfloat16, tag="o", name=f"ob{s_}_{bc2_}"
                        )
                        for g in range(2):
                            po = po_ps.tile(
                                [128, 512], dt.float32, tag="po",
                                name=f"po{s_}_{bc2_}_{g}",
                            )
                            nc.tensor.matmul(
                                po,
                                en_ref[64 * g : 64 * g + 64,
                                       bc2_ * 128 : (bc2_ + 1) * 128],
                                e4_sb[64 * g : 64 * g + 64,
                                      s_ * 512 : (s_ + 1) * 512],
                                start=True,
                                stop=True,
                                tile_position=(64 * g, 0),
                            )
                            obh = ob[:, g * 512 : (g + 1) * 512]
                            if (2 * bc2_ + g) % EMB_ACT_MOD < EMB_ACT_LIM:
                                nc.scalar.copy(obh, po)
                            else:
                                nc.vector.tensor_copy(obh, po)
                        nc.sync.dma_start(out=out_r[bc2_, s_], in_=ob)

                    return emit_group

                for qb in (2 * c, 2 * c + 1):
                    for u in range(4):
                        pending_emb.append(make_group(s, 4 * qb + u, en_sb))
        emit(pending_z)
        emit(pending_tail)
        emit(pending_emb)

    nc.compile()
    return nc


def _host_pack(inputs):
    """Pack full f32 inputs into per-core bf16 device arrays."""
    x = np.ascontiguousarray(inputs["x"], dtype=np.float32)
    w1 = np.asarray(inputs["w1"], dtype=np.float32)
    b1 = np.asarray(inputs["b1"], dtype=np.float32)
    w2 = np.asarray(inputs["w2"], dtype=np.float32)
    b2 = np.asarray(inputs["b2"], dtype=np.float32)
    tau = np.asarray(inputs["tau"], dtype=np.float32)
    emb = np.asarray(inputs["emb"], dtype=np.float32)
    rws = [np.asarray(inputs[f"rw{l}"], dtype=np.float32) for l in range(3)]
    rbs = [np.asarray(inputs[f"rb{l}"], dtype=np.float32) for l in range(3)]

    eye = np.eye(D, dtype=np.float32)
    xT = np.concatenate([x, np.ones((B, F, 1), np.float32)], axis=2)
    xT = np.ascontiguousarray(xT.transpose(1, 2, 0))  # [F, 4, B]
    w1a = np.concatenate([w1, b1[:, None, :]], axis=1)  # [F, 4, D]

    in_maps = []
    for cidx in range(NCORES):
        f0 = cidx * FC
        xpk = np.zeros((NPAIR, 8, B), BF16)
        w1k = np.zeros((64, NPAIR, 128), BF16)
        rwk = np.zeros((128, 3, NPAIR, 2, 64), BF16)
        rbk = np.zeros((128, 3, NPAIR), np.float32)
        w2k = np.zeros((128, NPAIR, 32), BF16)
        b2k = np.zeros((128, NSTACK), np.float32)
        tauk = np.zeros((128, NSTACK), np.float32)
        # garbage partitions keep tau=0 so exp(0)=1 stays finite
        e4k = np.zeros((128, NSTACK, 512), BF16)
        for pr in range(NPAIR):
            fa, fb = f0 + 2 * pr, f0 + 2 * pr + 1
            odd = pr % 2
            xpk[pr, 0:4] = xT[fa]
            xpk[pr, 4:8] = xT[fb]
            x0 = 32 * odd
            w1k[x0 : x0 + 4, pr, 0:64] = w1a[fa]
            w1k[x0 + 4 : x0 + 8, pr, 64:128] = w1a[fb]
            for l in range(3):
                # even pairs: feature fi's 64x64 block sits at partition rows
                # 64*fi every layer.  odd pairs ping-pong: input rows for
                # fi at layer l are 64*((l+fi)%2).
                for fi, ff in ((0, fa), (1, fb)):
                    rin = 64 * fi if not odd else 64 * ((l + fi) % 2)
                    ro = 64 * fi if not odd else 64 * ((l + 1 + fi) % 2)
                    rwk[rin : rin + 64, l, pr, fi] = rws[l][ff] + eye
                    rbk[ro : ro + 64, l, pr] = rbs[l][ff]
            # after 3 residual layers an odd pair ends with feature a on the
            # upper partition half (parity 1), so its w2 contraction rows swap
            za = 64 * (0 if not odd else 1)
            w2k[za : za + 64, pr, 0:8] = w2[fa]
            w2k[64 - za : 128 - za, pr, 8:16] = w2[fb]
            s, jj = pr // 4, pr % 4
            g, pl = jj // 2, jj % 2
            for fi, ff in ((0, fa), (1, fb)):
                rows = slice(32 * jj + 8 * fi, 32 * jj + 8 * fi + 8)
                b2k[rows, s] = b2[ff]
                tauk[rows, s] = tau[ff]
                # emb moving block for the 2-pair matmul: stationary rows are
                # en[64g : 64g+64]; within that, pair pl's probs sit at
                # relative rows 32*pl + 8*fi; its emb block streams to output
                # cols pl*256 + fi*128.
                e4k[64 * g + 32 * pl + 8 * fi : 64 * g + 32 * pl + 8 * fi + 8,
                    s, 256 * pl + 128 * fi : 256 * pl + 128 * fi + 128] = emb[ff]
        # sum-over-k stationary with broadcast to all 128 rows; garbage
        # partitions duplicate the pair's second feature so values stay sane.
        ob = np.zeros((128, 128), BF16)
        for jj in range(4):
            for g in range(4):
                src = 32 * jj + 8 * min(g, 1)
                ob[src : src + 8, 32 * jj + 8 * g : 32 * jj + 8 * g + 8] = 1
        m = {
            "xp": xpk,
            "w1p": w1k.reshape(64, NPAIR * 128),
            "rwp": rwk.reshape(128, 3 * NPAIR * 128),
            "rbp": rbk.reshape(128, 3 * NPAIR),
            "w2p": w2k.reshape(128, NPAIR * 32),
            "b2s": b2k,
            "taus": tauk,
            "e4p": e4k.reshape(128, NSTACK * 512),
            "onesbd": ob,
        }
        in_maps.append(m)
    return in_maps


def _get_compiled():
    global _compiled
    if _compiled is None:
        _compiled = _build_bass()
    return _compiled


def run_on_hw(in_maps, trace=False):
    from concourse import bass_utils

    nc = _get_compiled()
    res = bass_utils.run_bass_kernel_spmd(
        nc, in_maps, core_ids=list(range(NCORES)), trace=trace
    )
    return res


def kernel(**inputs):
    in_maps = _host_pack(inputs)
    res = run_on_hw(in_maps, trace=False)
    outs = [np.asarray(res.results[c]["out"], dtype=np.float32) for c in range(NCORES)]
    return np.concatenate(outs, axis=1)


# revision 22
# speedup vs baseline: 1.2814x; 1.2814x over previous
"""AutoDisBucketEncoder Trainium2 kernel (8 NeuronCores, feature-sharded).

Math (per feature f, batch b):
  h = lrelu(x_aug @ w1_aug)            # bias folded via ones column
  h = lrelu(h @ (rw_l + I) + rb_l)     # x3, residual folded into weights
  z = lrelu(h @ w2 + b2)
  e = exp(z * tau)
  out = (e / sum_k e) @ emb

Layout: features sharded 32/core; each core packs 2 features per 128
partitions (block-diagonal weights), streams the full 2048 batch as the
matmul moving dim.  Softmax runs in [k, b] layout; the sum-over-k and its
broadcast back to 128 partitions are done by one ones-block matmul.  The
embedding matmul packs 2 pairs (4 features) per instruction: stationary =
normalized probs [64, 128-batch], moving = block-diagonal emb [64, 512]
with zeroed garbage stripes; the two pair-groups sit on different PE
row-halves so their matmuls run concurrently, and each PSUM tile lands in
[batch, 8-feature] layout so output DMAs write 2KB contiguous lines.

All leaky-relus use Prelu (parametric_relu), which shares the activation
table set with Exp — no ACT_TABLE_LOAD switching mid-kernel.
"""

import sys

sys.path.insert(0, "/opt/trn_rl_repo")

import numpy as np
import ml_dtypes
from contextlib import ExitStack

BF16 = ml_dtypes.bfloat16
B, F, D, K, E = 2048, 256, 64, 8, 128
NCORES = 8
FC = F // NCORES          # 32 features per core
NPAIR = FC // 2           # 16
NSTACK = NPAIR // 4       # 4 stacks of 4 pairs
NEG = 0.01                # leaky slope
HB = B // 2               # 1024 batch half-chunk (2 PSUM banks in f32)

# h-chunk evictions: idx % MOD >= LIM -> DVE custom-op path, else ACT.
# idx % 4 is the pair index within a step; odd pairs go to DVE so each
# group's two evictions run on different engines concurrently.
DVE_RES_MOD = 2
DVE_RES_LIM = 1
# emb-psum evictions: idx % 16 < 7 -> ACT copy, else DVE copy
EMB_ACT_MOD = 16
EMB_ACT_LIM = 7

_compiled = None
SIM_SAFE = False  # substitute Relu for Prelu so CoreSim can execute


def _register_leaky_bias():
    import numpy as np
    from concourse.dve_spec import Spec, Src0, C0, C1, maxx, lower
    from concourse.dve_ops import (
        DveOp, DveOpSpec, OPS, CUSTOM_DVE_SPECS, _SUB_OPCODE_FOR_NAME,
        _CUSTOM_DVE_ROW_BASE, has_src1,
    )

    if "LEAKY_BIAS_ANT" in CUSTOM_DVE_SPECS:
        return next(o for o in OPS if o.name == "LEAKY_BIAS_ANT")
    spec = Spec(
        body=maxx(Src0 + C0, (Src0 + C0) * C1),
        reference=lambda in0, in1, s0, s1, imm2: np.maximum(
            in0 + s0, (in0 + s0) * s1
        ).astype(np.float32),
    )
    row = _CUSTOM_DVE_ROW_BASE + len(OPS)
    shas = {}
    for ver in ("v3", "v4"):
        uops = lower(spec, ver=ver)
        shas[ver] = DveOpSpec(
            name="LEAKY_BIAS_ANT", opcode=row, uops=uops, rd1_en=has_src1(spec)
        ).sha(ver)
    op = DveOp("LEAKY_BIAS_ANT", spec, subdim=False, uops_sha=shas)
    OPS.append(op)
    CUSTOM_DVE_SPECS[op.name] = spec
    _SUB_OPCODE_FOR_NAME[op.name] = row
    return op


def _build_bass():
    import concourse.bass as bass  # noqa: F401
    import concourse.mybir as mybir
    import concourse.tile as tile
    from concourse import bacc

    LEAKY_OP = _register_leaky_bias()

    dt = mybir.dt
    AF = mybir.ActivationFunctionType
    LRELU = AF.Relu if SIM_SAFE else AF.Prelu

    nc = bacc.Bacc("TRN2", target_bir_lowering=False, debug=False)

    xp = nc.dram_tensor("xp", [NPAIR, 8, B], dt.bfloat16, kind="ExternalInput").ap()
    w1p = nc.dram_tensor("w1p", [64, NPAIR * 128], dt.bfloat16, kind="ExternalInput").ap()
    rwp = nc.dram_tensor("rwp", [128, 3 * NPAIR * 128], dt.bfloat16, kind="ExternalInput").ap()
    rbp = nc.dram_tensor("rbp", [128, 3 * NPAIR], dt.float32, kind="ExternalInput").ap()
    w2p = nc.dram_tensor("w2p", [128, NPAIR * 32], dt.bfloat16, kind="ExternalInput").ap()
    b2s = nc.dram_tensor("b2s", [128, NSTACK], dt.float32, kind="ExternalInput").ap()
    taus = nc.dram_tensor("taus", [128, NSTACK], dt.float32, kind="ExternalInput").ap()
    onesbd = nc.dram_tensor("onesbd", [128, 128], dt.bfloat16, kind="ExternalInput").ap()
    e4p = nc.dram_tensor("e4p", [128, NSTACK * 512], dt.bfloat16, kind="ExternalInput").ap()
    out = nc.dram_tensor("out", [B, FC * E], dt.bfloat16, kind="ExternalOutput").ap()

    with tile.TileContext(nc) as tc, ExitStack() as ctx:
        const = ctx.enter_context(tc.tile_pool(name="const", bufs=1))
        xpool = ctx.enter_context(tc.tile_pool(name="xpool", bufs=3))
        hpool = ctx.enter_context(tc.tile_pool(name="hpool", bufs=8))
        tpool = ctx.enter_context(tc.tile_pool(name="tpool", bufs=3))
        epool = ctx.enter_context(tc.tile_pool(name="epool", bufs=2))
        rpool = ctx.enter_context(tc.tile_pool(name="rpool", bufs=2))
        opool = ctx.enter_context(tc.tile_pool(name="opool", bufs=4))
        h_ps = ctx.enter_context(tc.tile_pool(name="h_ps", bufs=3, space="PSUM"))
        po_ps = ctx.enter_context(tc.tile_pool(name="po_ps", bufs=2, space="PSUM"))

        # ---- constants into SBUF (all host-side pre-transposed, contiguous) ----
        w1_sb = const.tile([64, NPAIR * 128], dt.bfloat16)
        nc.sync.dma_start(out=w1_sb, in_=w1p)
        rb_sb = const.tile([128, 3 * NPAIR], dt.float32)
        nc.sync.dma_start(out=rb_sb, in_=rbp)
        rw_sb = const.tile([128, 3 * NPAIR * 128], dt.bfloat16)
        for l in range(3):
            nc.sync.dma_start(
                out=rw_sb[:, l * NPAIR * 128 : (l + 1) * NPAIR * 128],
                in_=rwp[:, l * NPAIR * 128 : (l + 1) * NPAIR * 128],
            )
        w2_sb = const.tile([128, NPAIR * 32], dt.bfloat16)
        nc.sync.dma_start(out=w2_sb, in_=w2p)
        b2_sb = const.tile([128, NSTACK], dt.float32)
        nc.sync.dma_start(out=b2_sb, in_=b2s)
        tau_sb = const.tile([128, NSTACK], dt.float32)
        nc.sync.dma_start(out=tau_sb, in_=taus)
        ones_sb = const.tile([128, 128], dt.bfloat16)
        nc.sync.dma_start(out=ones_sb, in_=onesbd)
        e4_sb = const.tile([128, NSTACK * 512], dt.bfloat16)
        nc.sync.dma_start(out=e4_sb, in_=e4p)

        # out[b, fc*E] viewed as [bc(16), p(128), s(4), e(1024)]
        out_r = out.rearrange("(bc p) (s e) -> bc s p e", p=128, e=1024)

        def evict_h(idx, h, ph, rb_ap):
            """psum -> sbuf bf16 with (optional bias add and) leaky relu."""
            if idx % DVE_RES_MOD >= DVE_RES_LIM:
                nc.vector._custom_dve(
                    LEAKY_OP,
                    out=h,
                    in0=ph,
                    s0=0.0 if rb_ap is None else rb_ap,
                    s1=NEG,
                )
            else:
                if rb_ap is None:
                    nc.scalar.activation(h, ph, LRELU, alpha=NEG)
                else:
                    nc.scalar.activation(h, ph, LRELU, bias=rb_ap, alpha=NEG)

        res_idx = 0
        pending_z = []     # deferred z-epilogue (dual exp + sumexp mms)
        pending_tail = []  # deferred DVE softmax tail (recip/mul)
        pending_emb = []   # emb po-group closures

        def emit(lst, n=99):
            for _ in range(min(n, len(lst))):
                lst.pop(0)()

        for s in range(NSTACK):
            e_sb = epool.tile([128, B], dt.bfloat16, tag="e", name=f"e{s}")
            en_sb = epool.tile([128, B], dt.bfloat16, tag="en", name=f"en{s}")
            for c in range(2):
                # ---- h pipeline: pairs processed two at a time; the even
                # pair uses the diagonal 64x64 PE tiles and the odd pair the
                # off-diagonal ones, so each residual step runs 4 concurrent
                # matmuls (full PE array).  The odd pair's two features swap
                # partition halves every residual layer (ping-pong); host
                # packing of rw/rb/w2 accounts for it.  Previous chunk's
                # softmax/emb work drips in between steps. ----
                hs = [None] * 4
                for grp in range(2):
                    xt = xpool.tile(
                        [64, HB], dt.bfloat16, tag="x", name=f"x{s}_{grp}_{c}"
                    )
                    for jj in range(2):
                        p = 4 * s + 2 * grp + jj
                        # inputs go out on the GpSimd DMA queue so they never
                        # queue behind output DMAs that wait on evictions
                        nc.gpsimd.dma_start(
                            out=xt[32 * jj : 32 * jj + 8, :],
                            in_=xp[p][:, c * HB : (c + 1) * HB],
                        )
                    hs[2 * grp] = xt
                    hs[2 * grp + 1] = xt
                for step in range(4):  # 0: L1, 1-3: residual layers
                    for grp in range(2):
                        phs = [None, None]
                        for jj in range(2):
                            j = 2 * grp + jj
                            p = 4 * s + j
                            phs[jj] = h_ps.tile(
                                [128, HB], dt.float32, tag="h",
                                name=f"ph{p}_{c}_{step}",
                            )
                        if step == 0:
                            for q in range(2):
                                qs = slice(q * 512, (q + 1) * 512)
                                for jj in range(2):
                                    p = 4 * s + 2 * grp + jj
                                    r0 = 32 * jj
                                    nc.tensor.matmul(
                                        phs[jj][:, qs],
                                        w1_sb[r0 : r0 + 8, p * 128 : (p + 1) * 128],
                                        hs[2 * grp + jj][r0 : r0 + 8, qs],
                                        start=True,
                                        stop=True,
                                    )
                        else:
                            l = step - 1
                            for q in range(2):
                                qs = slice(q * 512, (q + 1) * 512)
                                for jj in range(2):
                                    p = 4 * s + 2 * grp + jj
                                    for fi in range(2):
                                        if jj == 0:
                                            rin = ro = 64 * fi
                                        else:
                                            rin = 64 * ((l + fi) % 2)
                                            ro = 64 * ((l + 1 + fi) % 2)
                                        off = ((l * NPAIR + p) * 2 + fi) * 64
                                        nc.tensor.matmul(
                                            phs[jj][ro : ro + 64, qs],
                                            rw_sb[rin : rin + 64, off : off + 64],
                                            hs[2 * grp + jj][rin : rin + 64, qs],
                                            start=True,
                                            stop=True,
                                        )
                        for jj in range(2):
                            j = 2 * grp + jj
                            p = 4 * s + j
                            rb_ap = (
                                None
                                if step == 0
                                else rb_sb[
                                    :, (step - 1) * NPAIR + p : (step - 1) * NPAIR + p + 1
                                ]
                            )
                            h2 = hpool.tile(
                                [128, HB], dt.bfloat16, tag="h",
                                name=f"h{p}_{c}_{step}",
                            )
                            evict_h(res_idx, h2, phs[jj], rb_ap)
                            res_idx += 1
                            hs[j] = h2
                    if step == 0:
                        emit(pending_z)
                    elif step == 1:
                        emit(pending_tail)
                    else:
                        emit(pending_emb, 3)
                pz = h_ps.tile([128, HB], dt.float32, tag="h", name=f"pz{s}_{c}")
                for q in range(2):
                    for j in range(4):
                        p = 4 * s + j
                        nc.tensor.matmul(
                            pz[32 * j : 32 * j + 32, q * 512 : (q + 1) * 512],
                            w2_sb[:, p * 32 : (p + 1) * 32],
                            hs[j][:, q * 512 : (q + 1) * 512],
                            start=True,
                            stop=True,
                            tile_position=(0, 32 * j),
                        )
                emit(pending_emb)

                def make_z(s_, c_, pz_ref, e_ref, en_ref, last_):
                    def z_chain():
                        t1 = tpool.tile(
                            [128, HB], dt.float32, tag="zt", name=f"t1_{s_}_{c_}"
                        )
                        nc.scalar.activation(
                            t1, pz_ref, LRELU,
                            bias=b2_sb[:, s_ : s_ + 1], alpha=NEG,
                        )
                        ev = e_ref[:, c_ * HB : (c_ + 1) * HB]
                        nc.scalar.activation(
                            ev, t1, AF.Exp, scale=tau_sb[:, s_ : s_ + 1]
                        )
                        ps_sum = h_ps.tile(
                            [128, HB], dt.float32, tag="h", name=f"psum{s_}_{c_}"
                        )
                        for q in range(2):
                            nc.tensor.matmul(
                                ps_sum[:, q * 512 : (q + 1) * 512],
                                ones_sb,
                                ev[:, q * 512 : (q + 1) * 512],
                                start=True,
                                stop=True,
                            )

                        def tail():
                            rcf = rpool.tile(
                                [128, HB], dt.float32, tag="rcf", name=f"rcf{s_}_{c_}"
                            )
                            nc.vector.reciprocal_approx_fast(out=rcf, in_=ps_sum)
                            eng = nc.vector if last_ else nc.gpsimd
                            eng.tensor_mul(
                                en_ref[:, c_ * HB : (c_ + 1) * HB], ev, rcf
                            )

                        pending_tail.append(tail)

                    return z_chain

                pending_z.append(
                    make_z(s, c, pz, e_sb, en_sb, s == NSTACK - 1 and c == 1)
                )

                def make_group(s_, bc2_, en_ref):
                    def emit_group():
                        ob = opool.tile(
                            [128, 1024], dt.bfloat16, tag="o", name=f"ob{s_}_{bc2_}"
                        )
                        for g in range(2):
                            po = po_ps.tile(
                                [128, 512], dt.float32, tag="po",
                                name=f"po{s_}_{bc2_}_{g}",
                            )
                            nc.tensor.matmul(
                                po,
                                en_ref[64 * g : 64 * g + 64,
                                       bc2_ * 128 : (bc2_ + 1) * 128],
                                e4_sb[64 * g : 64 * g + 64,
                                      s_ * 512 : (s_ + 1) * 512],
                                start=True,
                                stop=True,
                                tile_position=(64 * g, 0),
                            )
                            obh = ob[:, g * 512 : (g + 1) * 512]
                            if (2 * bc2_ + g) % EMB_ACT_MOD < EMB_ACT_LIM:
                                nc.scalar.copy(obh, po)
                            else:
                                nc.vector.tensor_copy(obh, po)
                        nc.sync.dma_start(out=out_r[bc2_, s_], in_=ob)

                    return emit_group

                for qb in (2 * c, 2 * c + 1):
                    for u in range(4):
                        pending_emb.append(make_group(s, 4 * qb + u, en_sb))
        emit(pending_z)
        emit(pending_tail)
        emit(pending_emb)

    nc.compile()
    return nc


def _host_pack(inputs):
    """Pack full f32 inputs into per-core bf16 device arrays."""
    x = np.ascontiguousarray(inputs["x"], dtype=np.float32)
    w1 = np.asarray(inputs["w1"], dtype=np.float32)
    b1 = np.asarray(inputs["b1"], dtype=np.float32)
    w2 = np.asarray(inputs["w2"], dtype=np.float32)
    b2 = np.asarray(inputs["b2"], dtype=np.float32)
    tau = np.asarray(inputs["tau"], dtype=np.float32)
    emb = np.asarray(inputs["emb"], dtype=np.float32)
    rws = [np.asarray(inputs[f"rw{l}"], dtype=np.float32) for l in range(3)]
    rbs = [np.asarray(inputs[f"rb{l}"], dtype=np.float32) for l in range(3)]

    eye = np.eye(D, dtype=np.float32)
    xT = np.concatenate([x, np.ones((B, F, 1), np.float32)], axis=2)
    xT = np.ascontiguousarray(xT.transpose(1, 2, 0))  # [F, 4, B]
    w1a = np.concatenate([w1, b1[:, None, :]], axis=1)  # [F, 4, D]

    in_maps = []
    for cidx in range(NCORES):
        f0 = cidx * FC
        xpk = np.zeros((NPAIR, 8, B), BF16)
        w1k = np.zeros((64, NPAIR, 128), BF16)
        rwk = np.zeros((128, 3, NPAIR, 2, 64), BF16)
        rbk = np.zeros((128, 3, NPAIR), np.float32)
        w2k = np.zeros((128, NPAIR, 32), BF16)
        b2k = np.zeros((128, NSTACK), np.float32)
        tauk = np.zeros((128, NSTACK), np.float32)
        # garbage partitions keep tau=0 so exp(0)=1 stays finite
        e4k = np.zeros((128, NSTACK, 512), BF16)
        for pr in range(NPAIR):
            fa, fb = f0 + 2 * pr, f0 + 2 * pr + 1
            odd = pr % 2
            xpk[pr, 0:4] = xT[fa]
            xpk[pr, 4:8] = xT[fb]
            x0 = 32 * odd
            w1k[x0 : x0 + 4, pr, 0:64] = w1a[fa]
            w1k[x0 + 4 : x0 + 8, pr, 64:128] = w1a[fb]
            for l in range(3):
                # even pairs: feature fi's 64x64 block sits at partition rows
                # 64*fi every layer.  odd pairs ping-pong: input rows for
                # fi at layer l are 64*((l+fi)%2).
                for fi, ff in ((0, fa), (1, fb)):
                    rin = 64 * fi if not odd else 64 * ((l + fi) % 2)
                    ro = 64 * fi if not odd else 64 * ((l + 1 + fi) % 2)
                    rwk[rin : rin + 64, l, pr, fi] = rws[l][ff] + eye
                    rbk[ro : ro + 64, l, pr] = rbs[l][ff]
            # after 3 residual layers an odd pair ends with feature a on the
            # upper partition half (parity 1), so its w2 contraction rows swap
            za = 64 * (0 if not odd else 1)
            w2k[za : za + 64, pr, 0:8] = w2[fa]
            w2k[64 - za : 128 - za, pr, 8:16] = w2[fb]
            s, jj = pr // 4, pr % 4
            g, pl = jj // 2, jj % 2
            for fi, ff in ((0, fa), (1, fb)):
                rows = slice(32 * jj + 8 * fi, 32 * jj + 8 * fi + 8)
                b2k[rows, s] = b2[ff]
                tauk[rows, s] = tau[ff]
                # emb moving block for the 2-pair matmul: stationary rows are
                # en[64g : 64g+64]; within that, pair pl's probs sit at
                # relative rows 32*pl + 8*fi; its emb block streams to output
                # cols pl*256 + fi*128.
                e4k[64 * g + 32 * pl + 8 * fi : 64 * g + 32 * pl + 8 * fi + 8,
                    s, 256 * pl + 128 * fi : 256 * pl + 128 * fi + 128] = emb[ff]
        # sum-over-k stationary with broadcast to all 128 rows; garbage
        # partitions duplicate the pair's second feature so values stay sane.
        ob = np.zeros((128, 128), BF16)
        for jj in range(4):
            for g in range(4):
                src = 32 * jj + 8 * min(g, 1)
                ob[src : src + 8, 32 * jj + 8 * g : 32 * jj + 8 * g + 8] = 1
        m = {
            "xp": xpk,
            "w1p": w1k.reshape(64, NPAIR * 128),
            "rwp": rwk.reshape(128, 3 * NPAIR * 128),
            "rbp": rbk.reshape(128, 3 * NPAIR),
            "w2p": w2k.reshape(128, NPAIR * 32),
            "b2s": b2k,
            "taus": tauk,
            "e4p": e4k.reshape(128, NSTACK * 512),
            "onesbd": ob,
        }
        in_maps.append(m)
    return in_maps


def _get_compiled():
    global _compiled
    if _compiled is None:
        _compiled = _build_bass()
    return _compiled


def run_on_hw(in_maps, trace=False):
    from concourse import bass_utils

    nc = _get_compiled()
    res = bass_utils.run_bass_kernel_spmd(
        nc, in_maps, core_ids=list(range(NCORES)), trace=trace
    )
    return res


def kernel(**inputs):
    in_maps = _host_pack(inputs)
    res = run_on_hw(in_maps, trace=False)
    outs = [np.asarray(res.results[c]["out"], dtype=np.float32) for c in range(NCORES)]
    return np.concatenate(outs, axis=1)
